# revision 2
# baseline (speedup 1.0000x reference)
"""AttentionPooler Trainium2 kernel (raw bacc, hand-synchronized pipeline).

Computes, per batch b:
    scores = feats[b] @ weight ; attn = softmax(scores) ; out[b] = attn @ feats[b]

Sharding: batch-parallel across 8 NeuronCores (batch b -> core b); no
cross-core communication. Single pass over feats (memory-bound); softmax
without max-subtraction (scores for this problem's distribution are bounded
by |s| < ~90 so exp() stays in f32 range; softmax is shift-invariant so the
result matches the reference). Weighted sums run on the PE in f32r; scores
use the fused DVE scalar_tensor_tensor with accum_out.

Design: the 32 MiB/core feats read is the roofline (~94 us at the ~358 GB/s
HBM-per-NC share), so everything is organized to keep the two HWDGE DMA
rings (sync + scalar) streaming wall-to-wall:

  * a flat SBUF arena holds 48 of the 64 row-blocks, so 10 of the 13
    transfers are issued unconditionally at t=0 (descriptor-level
    issue-ahead); only the 3 wrap-around transfers gate on PE retiring the
    arena region they overwrite, and those gates resolve ~40 us before the
    stream reaches them.
  * body transfers are 8 row-blocks (4 MiB) to amortize per-transfer
    completion latency; the two rings alternate chunks so one ring's
    completion stall hides under the other's packets.
  * the score/exp/matmul rings are deep (S=8) so the DVE never throttles on
    exp/PE and trails DMA arrivals closely; tail chunks shrink to [2,1,1]
    blocks so the post-stream drain is only the last block's
    score+exp+matmul+reciprocal (~3 us).

Every DVE op carries a free field-update of sem_dve (cumulative op count)
and a free always-satisfied field-wait on its predecessor; likewise every
PE matmul chains through sem_mm. These encode same-engine program order for
the race detector at zero hardware cost; cross-engine dependencies use the
standalone waits.

Pipeline (per chunk i of G_i row-blocks):
  sync/ACT : dma arena[o_i] <- feats rows        (wrap chunks wait PE gate)
  DVE      : G_i x scalar_tensor_tensor -> s[i%S]  (waits chunk-i dma)
  ACT      : p[i%S] = exp(s[i%S]), zg = rowsum     (waits dve i, pe i-S)
  PE       : acc += p.T @ f ; zacc += zg.T @ ones  (waits exp i)
tail:
  DVE  : rec = 1/zacc ; res = acc * rec          (waits pe all)
  sync : dma out <- res ; wait it out
"""

import contextlib

import numpy as np

import concourse.bass as bass
import concourse.bacc as bacc
from concourse import mybir
from concourse.bass_utils import run_bass_kernel_spmd

B = 8
N = 8192
D = 1024
P = 128

F32 = mybir.dt.float32
F32R = mybir.dt.float32r

W = 48  # arena capacity in 128-row blocks (192 KiB/partition)
S = 8  # s/p ring depth

_cache = {}


def _sched_sizes(nblocks):
    if nblocks == 64:
        return [2, 2, 4, 8, 8, 8, 8, 8, 8, 4, 2, 1, 1]
    sizes = []
    rem = nblocks
    for g in (2, 2, 4):
        if rem > g:
            sizes.append(g)
            rem -= g
    while rem > 8:
        sizes.append(8)
        rem -= 8
    if rem > 2:
        sizes.append(rem - 2)
        rem = 2
    while rem > 0:
        sizes.append(1)
        rem -= 1
    return sizes


def _layout(nblocks):
    """Chunk sizes, arena block-offsets, and reuse gates.

    gates[i] = largest chunk index whose arena region chunk i overwrites
    (-1 if the region is virgin); the DMA for chunk i must wait until PE
    has retired that chunk (sem_mm >= mmcum[gates[i]]).
    """
    sizes = _sched_sizes(nblocks)
    off, gates = [], []
    owner = [-1] * W
    o = 0
    for i, g in enumerate(sizes):
        assert g <= W
        if o + g > W:
            o = 0
        gate = -1
        for wb in range(o, o + g):
            gate = max(gate, owner[wb])
            owner[wb] = i
        assert gate < i
        off.append(o)
        gates.append(gate)
        o += g
        if o == W:
            o = 0
    return sizes, off, gates


def build(n=N, d=D):
    key = (n, d)
    if key in _cache:
        return _cache[key]

    nblocks = n // P
    assert nblocks * P == n
    nbank = d // 512
    sizes, off, gates = _layout(nblocks)
    nchunk = len(sizes)

    # cumulative counters after each chunk
    sttcum = []
    mmcum = []
    t_s, t_m = 0, 0
    for g in sizes:
        t_s += g
        t_m += g * nbank + 1
        sttcum.append(t_s)
        mmcum.append(t_m)

    nc = bacc.Bacc("TRN2", target_bir_lowering=False, debug=False, num_devices=B)
    feats = nc.declare_dram_parameter("feats", [n, d], F32, isOutput=False)
    weight = nc.declare_dram_parameter("weight", [d], F32, isOutput=False)
    out = nc.declare_dram_parameter("out", [1, d], F32, isOutput=True)

    feats_f = feats.ap()
    srcs = []
    r0 = 0
    for g in sizes:
        rows = P * g
        srcs.append(
            feats_f[r0 : r0 + rows, :]
            .rearrange("(p g) d -> p (g d)", g=g)
            .bitcast(F32R)
        )
        r0 += rows

    w_ap = weight.ap()
    w_src = bass.AP(tensor=w_ap.tensor, offset=w_ap.offset, ap=[[0, P], w_ap.ap[0]])

    with contextlib.ExitStack() as ctx:
        arena = ctx.enter_context(nc.sbuf_tensor("arena", [P, W * d], F32R))
        scr = [
            ctx.enter_context(nc.sbuf_tensor(f"scr{k}", [P, d], F32)) for k in range(2)
        ]
        w_bc = ctx.enter_context(nc.sbuf_tensor("w_bc", [P, d], F32))
        gmax = max(sizes)
        s_t = [
            ctx.enter_context(nc.sbuf_tensor(f"s{k}", [P, gmax], F32)) for k in range(S)
        ]
        p_t = [
            ctx.enter_context(nc.sbuf_tensor(f"p{k}", [P, gmax], F32R))
            for k in range(S)
        ]
        zg = [
            ctx.enter_context(nc.sbuf_tensor(f"zg{k}", [P, 1], F32)) for k in range(S)
        ]
        ones = ctx.enter_context(nc.sbuf_tensor("ones", [P, 1], F32))
        rec = ctx.enter_context(nc.sbuf_tensor("rec", [1, 1], F32))
        # final result reuses scr[0]'s partition-0 row (scr is dead by then)
        res = scr[0][0:1, :]
        acc = ctx.enter_context(nc.psum_tensor("acc", [1, d], F32))
        zacc = ctx.enter_context(nc.psum_tensor("zacc", [1, 1], F32))

        block = ctx.enter_context(nc.Block())
        sem_wb = ctx.enter_context(nc.semaphore("sem_wb"))
        sem_c = [
            ctx.enter_context(nc.semaphore(f"sem_c{i}")) for i in range(nchunk)
        ]
        sem_out = ctx.enter_context(nc.semaphore("sem_out"))
        sem_dve = ctx.enter_context(nc.semaphore("sem_dve"))
        sem_exp = ctx.enter_context(nc.semaphore("sem_exp"))
        sem_mm = ctx.enter_context(nc.semaphore("sem_mm"))
        sem_res = ctx.enter_context(nc.semaphore("sem_res"))
        sem_one = ctx.enter_context(nc.semaphore("sem_one"))

        def chunk_dst(i):
            return arena[:, off[i] * d : (off[i] + sizes[i]) * d]

        def fblk(i, gg):
            return arena[:, (off[i] + gg) * d : (off[i] + gg + 1) * d]

        # odd (scalar-ring) chunks that must gate on PE retiring arena space
        # are issued from inside the exp loop, right after exp(place) where
        # the gate has long been satisfied; everything else is issued
        # unconditionally up front.
        odd_gated = {}
        for i in range(1, nchunk, 2):
            if gates[i] >= 0:
                place = min(gates[i] + 2, i - 1)
                odd_gated.setdefault(place, []).append(i)

        @block.sync
        def _(sync):
            watermark = -1
            for i in range(0, nchunk, 2):
                if gates[i] > watermark:
                    sync.wait_ge(sem_mm, mmcum[gates[i]])
                    watermark = gates[i]
                sync.dma_start(out=chunk_dst(i), in_=srcs[i]).then_inc(sem_c[i], 16)
            sync.wait_ge(sem_res, 1)
            sync.dma_start(out=out[:], in_=res).then_inc(sem_out, 16)
            sync.wait_ge(sem_out, 16)

        @block.vector
        def _(vector):
            nc.vector.memset(ones[:], 1.0).then_inc(sem_one, 1)
            vector.wait_ge(sem_wb, 16)
            kop = 0
            for i, g in enumerate(sizes):
                vector.wait_ge(sem_c[i], 16)
                if i >= S:
                    vector.wait_ge(sem_exp, i - S + 1)
                s = s_t[i % S]
                for gg in range(g):
                    ins = nc.vector.scalar_tensor_tensor(
                        out=scr[kop % 2][:],
                        in0=fblk(i, gg).bitcast(F32),
                        scalar=1.0,
                        in1=w_bc[:],
                        op0=mybir.AluOpType.mult,
                        op1=mybir.AluOpType.mult,
                        accum_out=s[:, gg : gg + 1],
                    )
                    ins.then_inc(sem_dve, 1)
                    if kop >= 1:
                        ins._wait_ge(sem_dve, kop - 1)
                    kop += 1
            vector.wait_ge(sem_mm, mmcum[-1])
            r1 = nc.vector.reciprocal(rec[:], zacc[:])
            r1.then_inc(sem_dve, 1)
            r1._wait_ge(sem_dve, kop - 1)
            r2 = nc.vector.tensor_scalar_mul(res, acc[:], rec[:])
            r2.then_inc(sem_res, 1)
            r2._wait_ge(sem_dve, kop + 1)

        @block.scalar
        def _(scalar):
            scalar.dma_start(out=w_bc[:], in_=w_src).then_inc(sem_wb, 16)
            for i in range(1, nchunk, 2):
                if gates[i] < 0:
                    scalar.dma_start(out=chunk_dst(i), in_=srcs[i]).then_inc(
                        sem_c[i], 16
                    )
            watermark = -1
            for i, g in enumerate(sizes):
                scalar.wait_ge(sem_dve, sttcum[i])
                if i >= S:
                    scalar.wait_ge(sem_mm, mmcum[i - S])
                nc.scalar.activation(
                    p_t[i % S][:, 0:g],
                    s_t[i % S][:, 0:g],
                    mybir.ActivationFunctionType.Exp,
                    accum_out=zg[i % S][:],
                ).then_inc(sem_exp, 1)
                for j in odd_gated.get(i, ()):
                    if gates[j] > watermark:
                        scalar.wait_ge(sem_mm, mmcum[gates[j]])
                        watermark = gates[j]
                    scalar.dma_start(out=chunk_dst(j), in_=srcs[j]).then_inc(
                        sem_c[j], 16
                    )

        @block.tensor
        def _(tensor):
            tensor.wait_ge(sem_one, 1)
            mop = 0
            for i, g in enumerate(sizes):
                tensor.wait_ge(sem_exp, i + 1)
                p = p_t[i % S]
                for gg in range(g):
                    first = i == 0 and gg == 0
                    last = i == nchunk - 1 and gg == g - 1
                    f = fblk(i, gg)
                    for bk in range(nbank):
                        ins = nc.tensor.matmul(
                            acc[:, bk * 512 : (bk + 1) * 512],
                            p[:, gg : gg + 1],
                            f[:, bk * 512 : (bk + 1) * 512],
                            start=first,
                            stop=last,
                        )
                        ins.then_inc(sem_mm, 1)
                        if mop >= 1:
                            ins._wait_ge(sem_mm, mop - 1)
                        mop += 1
                ins = nc.tensor.matmul(
                    zacc[:],
                    zg[i % S][:],
                    ones[:],
                    start=(i == 0),
                    stop=(i == nchunk - 1),
                )
                ins.then_inc(sem_mm, 1)
                ins._wait_ge(sem_mm, mop - 1)
                mop += 1

    nc.compile()
    _cache[key] = nc
    return nc


def kernel(feats, weight):
    feats = np.ascontiguousarray(np.asarray(feats), dtype=np.float32)
    weight = np.ascontiguousarray(np.asarray(weight), dtype=np.float32)
    assert feats.shape == (B, N, D) and weight.shape == (D,)
    nc = build()
    in_maps = [
        {"feats": np.ascontiguousarray(feats[b]), "weight": weight} for b in range(B)
    ]
    r = run_bass_kernel_spmd(nc, in_maps, core_ids=list(range(B)))
    return np.stack([r.results[b]["out"][0] for b in range(B)], axis=0)


if __name__ == "__main__":
    from concourse.bass_interp import CoreSim

    n_s, d_s = 2048, 1024
    nc = build(n=n_s, d=d_s)
    rng = np.random.default_rng(0)
    f = rng.standard_normal((n_s, d_s), dtype=np.float32)
    w = rng.random(d_s, dtype=np.float32)
    sim = CoreSim(nc, trace=False)
    sim.tensor("feats")[:] = f
    sim.tensor("weight")[:] = w
    sim.simulate(check_with_hw=False)
    got = np.array(sim.tensor("out"))[0]

    s = (f.astype(np.float64) * w.astype(np.float64)).sum(1)
    p = np.exp(s - s.max())
    exp = (p / p.sum()) @ f.astype(np.float64)
    rel = np.abs(got - exp).max() / np.abs(exp).max()
    print("CoreSim rel err:", rel)
    assert rel < 2e-3, rel
    print("SMOKE OK")


# revision 3
# speedup vs baseline: 1.1204x; 1.1204x over previous
"""AttentionPooler Trainium2 kernel (raw bacc, hand-synchronized pipeline).

Computes, per batch b:
    scores = feats[b] @ weight ; attn = softmax(scores) ; out[b] = attn @ feats[b]

Sharding: batch-parallel across 8 NeuronCores (batch b -> core b); no
cross-core communication. Single pass over feats (memory-bound); softmax
without max-subtraction (scores for this problem's distribution are bounded
by |s| < ~90 so exp() stays in f32 range; softmax is shift-invariant so the
result matches the reference). Weighted sums run on the PE in f32r; scores
use the fused DVE scalar_tensor_tensor with accum_out.

Design notes (from trace analysis):

  * The 32 MiB/core feats read is the roofline (~94 us at the ~358 GB/s
    HBM-per-NC share). All transfers are issued as early as possible so the
    two HWDGE rings (sync + scalar) stream wall-to-wall: a flat SBUF arena
    holds 48 of the 64 row-blocks, so the first 24 transfers have no
    dependencies at all; the 10 wrap-around transfers gate on PE retiring
    the arena region they overwrite (resolved ~40 us before the stream
    reaches them) and live on the sync ring only, so the scalar ring's
    exp stream is never blocked behind a descriptor-generation stall.
  * The DVE score pass (~73 us serial) is co-critical with the stream, and
    scores wait for FULL-chunk arrival on a HALF-rate ring. Transfers are
    therefore small (2 row-blocks = 1 MiB; 1-block tail) so arrivals are
    fine-grained and the post-stream drain is just the last block's
    score+exp+matmul+reciprocal.
  * weight is replicated to [128, d] on the HOST: a [0,128]-stride DMA
    broadcast of the raw [d] vector re-reads the same HBM line 128 times
    and crawls (~17 us, measured), starving the ring behind it; a plain
    512 KiB read streams at line rate.
  * The final 1/z scaling of the [1, d] pooled vector runs on a single
    partition (~1 lane); it is split in half across DVE and ACT.

Every DVE op carries a free field-update of sem_dve (cumulative op count)
and a free always-satisfied field-wait on its predecessor; likewise every
PE matmul chains through sem_mm. These encode same-engine program order for
the race detector at zero hardware cost; cross-engine dependencies use the
standalone waits.

Pipeline (per chunk i of G_i row-blocks):
  sync/ACT : dma arena[o_i] <- feats rows        (wrap chunks wait PE gate)
  DVE      : G_i x scalar_tensor_tensor -> s[i%S]  (waits chunk-i dma)
  ACT      : p[i%S] = exp(s[i%S]), zg = rowsum     (waits dve i, pe i-S)
  PE       : acc += p.T @ f ; zacc += zg.T @ ones  (waits exp i)
tail:
  DVE  : rec = 1/zacc ; res_lo = acc_lo * rec    (waits pe all)
  ACT  : res_hi = acc_hi * rec                   (waits recip)
  sync : dma out <- res ; wait it out
"""

import contextlib

import numpy as np

import concourse.bass as bass
import concourse.bacc as bacc
from concourse import mybir
from concourse.bass_utils import run_bass_kernel_spmd

B = 8
N = 8192
D = 1024
P = 128

F32 = mybir.dt.float32
F32R = mybir.dt.float32r

W = 48  # arena capacity in 128-row blocks (192 KiB/partition)
S = 8  # s/p ring depth

_cache = {}


def _layout(nblocks):
    """Chunk sizes, arena offsets, reuse gates, and ring assignment.

    gates[i] = largest chunk index whose arena region chunk i overwrites
    (-1 if the region is virgin); the DMA for chunk i must wait until PE
    has retired that chunk (sem_mm >= mmcum[gates[i]]).

    on_sync[i]: gated chunks all go on the sync ring (the scalar ring's
    program interleaves exp ops, which must never queue behind a gated
    descriptor-generation stall); ungated chunks split 1:2 sync:scalar so
    both rings carry ~half the bytes and arrivals track chunk order.
    """
    sizes = [2] * (nblocks // 2) + [1] * (nblocks % 2)
    if nblocks >= 8:
        # replace the last two 2-block chunks with four 1-block chunks
        sizes = sizes[:-2] + [1, 1, 1, 1]
    assert sum(sizes) == nblocks
    off, gates = [], []
    owner = [-1] * W
    o = 0
    for i, g in enumerate(sizes):
        assert g <= W
        if o + g > W:
            o = 0
        gate = -1
        for wb in range(o, o + g):
            gate = max(gate, owner[wb])
            owner[wb] = i
        assert gate < i
        off.append(o)
        gates.append(gate)
        o += g
        if o == W:
            o = 0
    on_sync = []
    nu = 0
    for i in range(len(sizes)):
        if gates[i] >= 0:
            on_sync.append(True)
        else:
            on_sync.append(nu % 3 == 0)
            nu += 1
    return sizes, off, gates, on_sync


def build(n=N, d=D):
    key = (n, d)
    if key in _cache:
        return _cache[key]

    nblocks = n // P
    assert nblocks * P == n
    nbank = d // 512
    sizes, off, gates, on_sync = _layout(nblocks)
    nchunk = len(sizes)

    # cumulative counters after each chunk
    sttcum = []
    mmcum = []
    t_s, t_m = 0, 0
    for g in sizes:
        t_s += g
        t_m += g * nbank + 1
        sttcum.append(t_s)
        mmcum.append(t_m)

    nc = bacc.Bacc("TRN2", target_bir_lowering=False, debug=False, num_devices=B)
    feats = nc.declare_dram_parameter("feats", [n, d], F32, isOutput=False)
    weight = nc.declare_dram_parameter("weight", [P, d], F32, isOutput=False)
    out = nc.declare_dram_parameter("out", [1, d], F32, isOutput=True)

    feats_f = feats.ap()
    srcs = []
    r0 = 0
    for g in sizes:
        rows = P * g
        srcs.append(
            feats_f[r0 : r0 + rows, :]
            .rearrange("(p g) d -> p (g d)", g=g)
            .bitcast(F32R)
        )
        r0 += rows

    with contextlib.ExitStack() as ctx:
        arena = ctx.enter_context(nc.sbuf_tensor("arena", [P, W * d], F32R))
        scr = [
            ctx.enter_context(nc.sbuf_tensor(f"scr{k}", [P, d], F32)) for k in range(2)
        ]
        w_bc = ctx.enter_context(nc.sbuf_tensor("w_bc", [P, d], F32))
        gmax = max(sizes)
        s_t = [
            ctx.enter_context(nc.sbuf_tensor(f"s{k}", [P, gmax], F32)) for k in range(S)
        ]
        p_t = [
            ctx.enter_context(nc.sbuf_tensor(f"p{k}", [P, gmax], F32R))
            for k in range(S)
        ]
        zg = [
            ctx.enter_context(nc.sbuf_tensor(f"zg{k}", [P, 1], F32)) for k in range(S)
        ]
        ones = ctx.enter_context(nc.sbuf_tensor("ones", [P, 1], F32))
        rec = ctx.enter_context(nc.sbuf_tensor("rec", [1, 1], F32))
        # final result reuses scr[0]'s partition-0 row (scr is dead by then)
        res = scr[0][0:1, :]
        acc = ctx.enter_context(nc.psum_tensor("acc", [1, d], F32))
        zacc = ctx.enter_context(nc.psum_tensor("zacc", [1, 1], F32))

        block = ctx.enter_context(nc.Block())
        sem_wb = ctx.enter_context(nc.semaphore("sem_wb"))
        sem_c = [ctx.enter_context(nc.semaphore(f"sem_c{i}")) for i in range(nchunk)]
        sem_out = ctx.enter_context(nc.semaphore("sem_out"))
        sem_dve = ctx.enter_context(nc.semaphore("sem_dve"))
        sem_exp = ctx.enter_context(nc.semaphore("sem_exp"))
        sem_mm = ctx.enter_context(nc.semaphore("sem_mm"))
        sem_res = ctx.enter_context(nc.semaphore("sem_res"))
        sem_one = ctx.enter_context(nc.semaphore("sem_one"))

        def chunk_dst(i):
            return arena[:, off[i] * d : (off[i] + sizes[i]) * d]

        def fblk(i, gg):
            return arena[:, (off[i] + gg) * d : (off[i] + gg + 1) * d]

        half = (d // 2 // 512) * 512 or d // 2

        @block.sync
        def _(sync):
            watermark = -1
            for i in range(nchunk):
                if not on_sync[i]:
                    continue
                if gates[i] > watermark:
                    sync.wait_ge(sem_mm, mmcum[gates[i]])
                    watermark = gates[i]
                sync.dma_start(out=chunk_dst(i), in_=srcs[i]).then_inc(sem_c[i], 16)
            sync.wait_ge(sem_res, 2)
            sync.dma_start(out=out[:], in_=res).then_inc(sem_out, 16)
            sync.wait_ge(sem_out, 16)

        @block.vector
        def _(vector):
            nc.vector.memset(ones[:], 1.0).then_inc(sem_one, 1)
            vector.wait_ge(sem_wb, 16)
            kop = 0
            for i, g in enumerate(sizes):
                vector.wait_ge(sem_c[i], 16)
                if i >= S:
                    vector.wait_ge(sem_exp, i - S + 1)
                s = s_t[i % S]
                for gg in range(g):
                    ins = nc.vector.scalar_tensor_tensor(
                        out=scr[kop % 2][:],
                        in0=fblk(i, gg).bitcast(F32),
                        scalar=1.0,
                        in1=w_bc[:],
                        op0=mybir.AluOpType.mult,
                        op1=mybir.AluOpType.mult,
                        accum_out=s[:, gg : gg + 1],
                    )
                    ins.then_inc(sem_dve, 1)
                    if kop >= 1:
                        ins._wait_ge(sem_dve, kop - 1)
                    kop += 1
            vector.wait_ge(sem_mm, mmcum[-1])
            r1 = nc.vector.reciprocal(rec[:], zacc[:])
            r1.then_inc(sem_dve, 1)
            r1._wait_ge(sem_dve, kop - 1)
            r2 = nc.vector.tensor_scalar_mul(
                res[:, 0:half], acc[:, 0:half], rec[:]
            )
            r2.then_inc(sem_res, 1)
            r2._wait_ge(sem_dve, kop + 1)

        @block.scalar
        def _(scalar):
            scalar.dma_start(out=w_bc[:], in_=weight.ap().bitcast(F32)).then_inc(
                sem_wb, 16
            )
            for i in range(nchunk):
                if not on_sync[i]:
                    scalar.dma_start(out=chunk_dst(i), in_=srcs[i]).then_inc(
                        sem_c[i], 16
                    )
            for i, g in enumerate(sizes):
                scalar.wait_ge(sem_dve, sttcum[i])
                if i >= S:
                    scalar.wait_ge(sem_mm, mmcum[i - S])
                nc.scalar.activation(
                    p_t[i % S][:, 0:g],
                    s_t[i % S][:, 0:g],
                    mybir.ActivationFunctionType.Exp,
                    accum_out=zg[i % S][:],
                ).then_inc(sem_exp, 1)
            # other half of the final 1/z scaling (DVE computes rec first)
            scalar.wait_ge(sem_dve, sttcum[-1] + 1)
            nc.scalar.activation(
                res[:, half:d],
                acc[:, half:d],
                mybir.ActivationFunctionType.Copy,
                scale=rec[:],
            ).then_inc(sem_res, 1)

        @block.tensor
        def _(tensor):
            tensor.wait_ge(sem_one, 1)
            mop = 0
            for i, g in enumerate(sizes):
                tensor.wait_ge(sem_exp, i + 1)
                p = p_t[i % S]
                for gg in range(g):
                    first = i == 0 and gg == 0
                    last = i == nchunk - 1 and gg == g - 1
                    f = fblk(i, gg)
                    for bk in range(nbank):
                        ins = nc.tensor.matmul(
                            acc[:, bk * 512 : (bk + 1) * 512],
                            p[:, gg : gg + 1],
                            f[:, bk * 512 : (bk + 1) * 512],
                            start=first,
                            stop=last,
                        )
                        ins.then_inc(sem_mm, 1)
                        if mop >= 1:
                            ins._wait_ge(sem_mm, mop - 1)
                        mop += 1
                ins = nc.tensor.matmul(
                    zacc[:],
                    zg[i % S][:],
                    ones[:],
                    start=(i == 0),
                    stop=(i == nchunk - 1),
                )
                ins.then_inc(sem_mm, 1)
                ins._wait_ge(sem_mm, mop - 1)
                mop += 1

    nc.compile()
    _cache[key] = nc
    return nc


def kernel(feats, weight):
    feats = np.ascontiguousarray(np.asarray(feats), dtype=np.float32)
    weight = np.ascontiguousarray(np.asarray(weight), dtype=np.float32)
    assert feats.shape == (B, N, D) and weight.shape == (D,)
    w_rep = np.ascontiguousarray(np.broadcast_to(weight[None, :], (P, D)))
    nc = build()
    in_maps = [
        {"feats": np.ascontiguousarray(feats[b]), "weight": w_rep} for b in range(B)
    ]
    r = run_bass_kernel_spmd(nc, in_maps, core_ids=list(range(B)))
    return np.stack([r.results[b]["out"][0] for b in range(B)], axis=0)


if __name__ == "__main__":
    from concourse.bass_interp import CoreSim

    n_s, d_s = 2048, 1024
    nc = build(n=n_s, d=d_s)
    rng = np.random.default_rng(0)
    f = rng.standard_normal((n_s, d_s), dtype=np.float32)
    w = rng.random(d_s, dtype=np.float32)
    sim = CoreSim(nc, trace=False)
    sim.tensor("feats")[:] = f
    sim.tensor("weight")[:] = np.broadcast_to(w[None, :], (P, d_s))
    sim.simulate(check_with_hw=False)
    got = np.array(sim.tensor("out"))[0]

    s = (f.astype(np.float64) * w.astype(np.float64)).sum(1)
    p = np.exp(s - s.max())
    exp = (p / p.sum()) @ f.astype(np.float64)
    rel = np.abs(got - exp).max() / np.abs(exp).max()
    print("CoreSim rel err:", rel)
    assert rel < 2e-3, rel
    print("SMOKE OK")


# revision 7
# speedup vs baseline: 1.1596x; 1.0350x over previous
"""AttentionPooler Trainium2 kernel (raw bacc, hand-synchronized pipeline).

Computes, per batch b:
    scores = feats[b] @ weight ; attn = softmax(scores) ; out[b] = attn @ feats[b]

Sharding: batch-parallel across 8 NeuronCores (batch b -> core b); no
cross-core communication. Single pass over feats (memory-bound); softmax
without max-subtraction (scores for this problem's distribution are bounded
by |s| < ~90 so exp() stays in f32 range; softmax is shift-invariant so the
result matches the reference). Weighted sums run on the PE in f32r; scores
use the fused DVE scalar_tensor_tensor with accum_out.

Design notes (from trace analysis):

  * The 32 MiB/core feats read is the roofline (~94 us at the ~358 GB/s
    HBM-per-NC share). All transfers are issued as early as possible so the
    two HWDGE rings (sync + scalar) stream wall-to-wall: a flat SBUF arena
    holds 48 of the 64 row-blocks, so the first 24 transfers have no
    dependencies at all; the 10 wrap-around transfers gate on PE retiring
    the arena region they overwrite (resolved ~40 us before the stream
    reaches them) and live on the sync ring only, so the scalar ring's
    exp stream is never blocked behind a descriptor-generation stall.
  * The DVE score pass (~73 us serial) is co-critical with the stream, and
    scores wait for FULL-chunk arrival on a HALF-rate ring. Transfers are
    therefore small (2 row-blocks = 1 MiB; 1-block tail) so arrivals are
    fine-grained and the post-stream drain is just the last block's
    score+exp+matmul+reciprocal.
  * weight is replicated to [128, d] on the HOST: a [0,128]-stride DMA
    broadcast of the raw [d] vector re-reads the same HBM line 128 times
    and crawls (~17 us, measured), starving the ring behind it; a plain
    512 KiB read streams at line rate.
  * The final 1/z scaling of the [1, d] pooled vector runs on a single
    partition (~1 lane); it is split in half across DVE and ACT.

Every DVE op carries a free field-update of sem_dve (cumulative op count)
and a free always-satisfied field-wait on its predecessor; likewise every
PE matmul chains through sem_mm. These encode same-engine program order for
the race detector at zero hardware cost; cross-engine dependencies use the
standalone waits.

Pipeline (per chunk i of G_i row-blocks):
  sync/ACT : dma arena[o_i] <- feats rows        (wrap chunks wait PE gate)
  DVE      : G_i x scalar_tensor_tensor -> s[i%S]  (waits chunk-i dma)
  ACT      : p[i%S] = exp(s[i%S]), zg = rowsum     (waits dve i, pe i-S)
  PE       : acc += p.T @ f ; zacc += zg.T @ ones  (waits exp i)
tail:
  DVE  : rec = 1/zacc ; res_lo = acc_lo * rec    (waits pe all)
  ACT  : res_hi = acc_hi * rec                   (waits recip)
  sync : dma out <- res ; wait it out
"""

import contextlib

import numpy as np

import concourse.bass as bass
import concourse.bacc as bacc
from concourse import mybir
from concourse.bass_utils import run_bass_kernel_spmd

B = 8
N = 8192
D = 1024
P = 128

F32 = mybir.dt.float32
F32R = mybir.dt.float32r

W = 48  # arena capacity in 128-row blocks (192 KiB/partition)
S = 8  # s/p ring depth
L = 6  # DMA issue lead over DVE score progress, in chunks

_cache = {}


def _sched_sizes(nblocks):
    # head small (early DVE start), 4-block body, fine-grained tail so the
    # post-stream score drain is one block
    sizes = [2, 2]
    rem = nblocks - 4
    while rem >= 12:
        sizes.append(4)
        rem -= 4
    if rem >= 10:
        sizes.append(rem - 8)
        rem = 8
    assert rem == 8, nblocks
    sizes += [2, 2, 1, 1, 1, 1]
    return sizes


def _layout(nblocks):
    """Chunk sizes, arena offsets, and reuse gates.

    gates[i] = largest chunk index whose arena region chunk i overwrites
    (-1 if the region is virgin); the DMA for chunk i must wait until PE
    has retired that chunk (sem_mm >= mmcum[gates[i]]).
    """
    sizes = _sched_sizes(nblocks)
    assert sum(sizes) == nblocks
    off, gates = [], []
    owner = [-1] * W
    o = 0
    for i, g in enumerate(sizes):
        assert g <= W
        if o + g > W:
            o = 0
        gate = -1
        for wb in range(o, o + g):
            gate = max(gate, owner[wb])
            owner[wb] = i
        assert gate < i
        off.append(o)
        gates.append(gate)
        o += g
        if o == W:
            o = 0
    return sizes, off, gates


def build(n=N, d=D):
    key = (n, d)
    if key in _cache:
        return _cache[key]

    nblocks = n // P
    assert nblocks * P == n
    nbank = d // 512
    sizes, off, gates = _layout(nblocks)
    nchunk = len(sizes)

    # cumulative counters after each chunk
    sttcum = []
    mmcum = []
    t_s, t_m = 0, 0
    for g in sizes:
        t_s += g
        t_m += g * nbank + 1
        sttcum.append(t_s)
        mmcum.append(t_m)

    nc = bacc.Bacc("TRN2", target_bir_lowering=False, debug=False, num_devices=B)
    feats = nc.declare_dram_parameter("feats", [n, d], F32, isOutput=False)
    weight = nc.declare_dram_parameter("weight", [P, d], F32, isOutput=False)
    out = nc.declare_dram_parameter("out", [1, d], F32, isOutput=True)

    feats_f = feats.ap()
    srcs = []
    r0 = 0
    for g in sizes:
        rows = P * g
        srcs.append(
            feats_f[r0 : r0 + rows, :]
            .rearrange("(p g) d -> p (g d)", g=g)
            .bitcast(F32R)
        )
        r0 += rows

    with contextlib.ExitStack() as ctx:
        arena = ctx.enter_context(nc.sbuf_tensor("arena", [P, W * d], F32R))
        scr = [
            ctx.enter_context(nc.sbuf_tensor(f"scr{k}", [P, d], F32)) for k in range(2)
        ]
        w_bc = ctx.enter_context(nc.sbuf_tensor("w_bc", [P, d], F32))
        gmax = max(sizes)
        s_t = [
            ctx.enter_context(nc.sbuf_tensor(f"s{k}", [P, gmax], F32)) for k in range(S)
        ]
        p_t = [
            ctx.enter_context(nc.sbuf_tensor(f"p{k}", [P, gmax], F32R))
            for k in range(S)
        ]
        zg = [
            ctx.enter_context(nc.sbuf_tensor(f"zg{k}", [P, 1], F32)) for k in range(S)
        ]
        ones = ctx.enter_context(nc.sbuf_tensor("ones", [P, 1], F32))
        rec = ctx.enter_context(nc.sbuf_tensor("rec", [1, 1], F32))
        # final result reuses scr[0]'s partition-0 row (scr is dead by then)
        res = scr[0][0:1, :]
        acc = ctx.enter_context(nc.psum_tensor("acc", [1, d], F32))
        zacc = ctx.enter_context(nc.psum_tensor("zacc", [1, 1], F32))

        block = ctx.enter_context(nc.Block())
        sem_wb = ctx.enter_context(nc.semaphore("sem_wb"))
        sem_c = [ctx.enter_context(nc.semaphore(f"sem_c{i}")) for i in range(nchunk)]
        sem_out = ctx.enter_context(nc.semaphore("sem_out"))
        sem_dve = ctx.enter_context(nc.semaphore("sem_dve"))
        sem_exp = ctx.enter_context(nc.semaphore("sem_exp"))
        sem_mm = ctx.enter_context(nc.semaphore("sem_mm"))
        sem_res = ctx.enter_context(nc.semaphore("sem_res"))
        sem_one = ctx.enter_context(nc.semaphore("sem_one"))

        def chunk_dst(i):
            return arena[:, off[i] * d : (off[i] + sizes[i]) * d]

        def fblk(i, gg):
            return arena[:, (off[i] + gg) * d : (off[i] + gg + 1) * d]

        half = (d // 2 // 512) * 512 or d // 2

        @block.sync
        def _(sync):
            # even chunks; issue paced L chunks ahead of DVE score progress
            # (deep free-running queues measurably unbalance the two rings'
            # HBM arbitration; DVE-paced issue keeps them even)
            watermark = -1
            for i in range(0, nchunk, 2):
                if i >= L:
                    sync.wait_ge(sem_dve, sttcum[i - L])
                if gates[i] > watermark:
                    sync.wait_ge(sem_mm, mmcum[gates[i]])
                    watermark = gates[i]
                sync.dma_start(out=chunk_dst(i), in_=srcs[i]).then_inc(sem_c[i], 16)
            sync.wait_ge(sem_res, 2)
            sync.dma_start(out=out[:], in_=res).then_inc(sem_out, 16)
            sync.wait_ge(sem_out, 16)

        @block.vector
        def _(vector):
            nc.vector.memset(ones[:], 1.0).then_inc(sem_one, 1)
            vector.wait_ge(sem_wb, 16)
            kop = 0
            for i, g in enumerate(sizes):
                vector.wait_ge(sem_c[i], 16)
                if i >= S:
                    vector.wait_ge(sem_exp, i - S + 1)
                s = s_t[i % S]
                for gg in range(g):
                    ins = nc.vector.scalar_tensor_tensor(
                        out=scr[kop % 2][:],
                        in0=fblk(i, gg).bitcast(F32),
                        scalar=1.0,
                        in1=w_bc[:],
                        op0=mybir.AluOpType.mult,
                        op1=mybir.AluOpType.mult,
                        accum_out=s[:, gg : gg + 1],
                    )
                    ins.then_inc(sem_dve, 1)
                    if kop >= 1:
                        ins._wait_ge(sem_dve, kop - 1)
                    kop += 1
            vector.wait_ge(sem_mm, mmcum[-1])
            r1 = nc.vector.reciprocal(rec[:], zacc[:])
            r1.then_inc(sem_dve, 1)
            r1._wait_ge(sem_dve, kop - 1)
            r2 = nc.vector.tensor_scalar_mul(
                res[:, 0:half], acc[:, 0:half], rec[:]
            )
            r2.then_inc(sem_res, 1)
            r2._wait_ge(sem_dve, kop + 1)

        @block.scalar
        def _(scalar):
            scalar.dma_start(out=w_bc[:], in_=weight.ap()).then_inc(sem_wb, 16)
            watermark = -1

            def issue(j):
                nonlocal watermark
                if j % 2 == 1 and j < nchunk:
                    if gates[j] > watermark:
                        scalar.wait_ge(sem_mm, mmcum[gates[j]])
                        watermark = gates[j]
                    scalar.dma_start(out=chunk_dst(j), in_=srcs[j]).then_inc(
                        sem_c[j], 16
                    )

            for j in range(min(L, nchunk)):
                issue(j)
            for i, g in enumerate(sizes):
                scalar.wait_ge(sem_dve, sttcum[i])
                if i >= S:
                    scalar.wait_ge(sem_mm, mmcum[i - S])
                nc.scalar.activation(
                    p_t[i % S][:, 0:g],
                    s_t[i % S][:, 0:g],
                    mybir.ActivationFunctionType.Exp,
                    accum_out=zg[i % S][:],
                ).then_inc(sem_exp, 1)
                # the L-lead gate for chunk i+L is exactly "score(i) done",
                # which the exp above just waited for: zero-stall issue point
                issue(i + L)
            # other half of the final 1/z scaling (DVE computes rec first)
            scalar.wait_ge(sem_dve, sttcum[-1] + 1)
            nc.scalar.activation(
                res[:, half:d],
                acc[:, half:d],
                mybir.ActivationFunctionType.Copy,
                scale=rec[:],
            ).then_inc(sem_res, 1)

        @block.tensor
        def _(tensor):
            tensor.wait_ge(sem_one, 1)
            mop = 0
            for i, g in enumerate(sizes):
                tensor.wait_ge(sem_exp, i + 1)
                p = p_t[i % S]
                for gg in range(g):
                    first = i == 0 and gg == 0
                    last = i == nchunk - 1 and gg == g - 1
                    f = fblk(i, gg)
                    for bk in range(nbank):
                        ins = nc.tensor.matmul(
                            acc[:, bk * 512 : (bk + 1) * 512],
                            p[:, gg : gg + 1],
                            f[:, bk * 512 : (bk + 1) * 512],
                            start=first,
                            stop=last,
                        )
                        ins.then_inc(sem_mm, 1)
                        if mop >= 1:
                            ins._wait_ge(sem_mm, mop - 1)
                        mop += 1
                ins = nc.tensor.matmul(
                    zacc[:],
                    zg[i % S][:],
                    ones[:],
                    start=(i == 0),
                    stop=(i == nchunk - 1),
                )
                ins.then_inc(sem_mm, 1)
                ins._wait_ge(sem_mm, mop - 1)
                mop += 1

    nc.compile()
    _cache[key] = nc
    return nc


def kernel(feats, weight):
    feats = np.ascontiguousarray(np.asarray(feats), dtype=np.float32)
    weight = np.ascontiguousarray(np.asarray(weight), dtype=np.float32)
    assert feats.shape == (B, N, D) and weight.shape == (D,)
    w_rep = np.ascontiguousarray(np.broadcast_to(weight[None, :], (P, D)))
    nc = build()
    in_maps = [
        {"feats": np.ascontiguousarray(feats[b]), "weight": w_rep} for b in range(B)
    ]
    r = run_bass_kernel_spmd(nc, in_maps, core_ids=list(range(B)))
    return np.stack([r.results[b]["out"][0] for b in range(B)], axis=0)


if __name__ == "__main__":
    from concourse.bass_interp import CoreSim

    n_s, d_s = 2048, 1024
    nc = build(n=n_s, d=d_s)
    rng = np.random.default_rng(0)
    f = rng.standard_normal((n_s, d_s), dtype=np.float32)
    w = rng.random(d_s, dtype=np.float32)
    sim = CoreSim(nc, trace=False)
    sim.tensor("feats")[:] = f
    sim.tensor("weight")[:] = np.broadcast_to(w[None, :], (P, d_s))
    sim.simulate(check_with_hw=False)
    got = np.array(sim.tensor("out"))[0]

    s = (f.astype(np.float64) * w.astype(np.float64)).sum(1)
    p = np.exp(s - s.max())
    exp = (p / p.sum()) @ f.astype(np.float64)
    rel = np.abs(got - exp).max() / np.abs(exp).max()
    print("CoreSim rel err:", rel)
    assert rel < 2e-3, rel
    print("SMOKE OK")


# revision 12
# speedup vs baseline: 1.3511x; 1.1651x over previous
"""AttentionPooler Trainium2 kernel (raw bacc, hand-synchronized pipeline).

Computes, per batch b:
    scores = feats[b] @ weight ; attn = softmax(scores) ; out[b] = attn @ feats[b]

Sharding: batch-parallel across 8 NeuronCores (batch b -> core b); no
cross-core communication. Single pass over feats (memory-bound); softmax
without max-subtraction (scores for this problem's distribution are bounded
by |s| < ~90 so exp() stays in f32 range; softmax is shift-invariant so the
result matches the reference). Weighted sums run on the PE in f32r; scores
use the fused DVE scalar_tensor_tensor with accum_out.

Design notes (from trace analysis):

  * The 32 MiB/core feats read is the roofline (~94 us at the ~358 GB/s
    HBM-per-NC share). All transfers are issued as early as possible so the
    two HWDGE rings (sync + scalar) stream wall-to-wall: a flat SBUF arena
    holds 48 of the 64 row-blocks, so the first 24 transfers have no
    dependencies at all; the 10 wrap-around transfers gate on PE retiring
    the arena region they overwrite (resolved ~40 us before the stream
    reaches them) and live on the sync ring only, so the scalar ring's
    exp stream is never blocked behind a descriptor-generation stall.
  * The DVE score pass (~73 us serial) is co-critical with the stream, and
    scores wait for FULL-chunk arrival on a HALF-rate ring. Transfers are
    therefore small (2 row-blocks = 1 MiB; 1-block tail) so arrivals are
    fine-grained and the post-stream drain is just the last block's
    score+exp+matmul+reciprocal.
  * weight is replicated to [128, d] on the HOST: a [0,128]-stride DMA
    broadcast of the raw [d] vector re-reads the same HBM line 128 times
    and crawls (~17 us, measured), starving the ring behind it; a plain
    512 KiB read streams at line rate.
  * The final 1/z scaling of the [1, d] pooled vector runs on a single
    partition (~1 lane); it is split in half across DVE and ACT.

Every DVE op carries a free field-update of sem_dve (cumulative op count)
and a free always-satisfied field-wait on its predecessor; likewise every
PE matmul chains through sem_mm. These encode same-engine program order for
the race detector at zero hardware cost; cross-engine dependencies use the
standalone waits.

Pipeline (per chunk i of G_i row-blocks):
  sync/ACT : dma arena[o_i] <- feats rows        (wrap chunks wait PE gate)
  DVE      : G_i x scalar_tensor_tensor -> s[i%S]  (waits chunk-i dma)
  ACT      : p[i%S] = exp(s[i%S]), zg = rowsum     (waits dve i, pe i-S)
  PE       : acc += p.T @ f ; zacc += zg.T @ ones  (waits exp i)
tail:
  DVE  : rec = 1/zacc ; res_lo = acc_lo * rec    (waits pe all)
  ACT  : res_hi = acc_hi * rec                   (waits recip)
  sync : dma out <- res ; wait it out
"""

import contextlib

import numpy as np

import concourse.bass as bass
import concourse.bacc as bacc
from concourse import mybir
from concourse.bass_utils import run_bass_kernel_spmd

B = 8
N = 8192
D = 1024
P = 128

F32 = mybir.dt.float32
F32R = mybir.dt.float32r

W = 48  # arena capacity in 128-row blocks (192 KiB/partition)
S = 8  # s/p ring depth
L = 6  # DMA issue lead over DVE score progress, in chunks

_cache = {}


def _sched_sizes(nblocks):
    # head small (early DVE start), 4-block body, fine-grained tail so the
    # post-stream score drain is one block
    sizes = [1, 1, 2]
    rem = nblocks - 4
    while rem > 8:
        sizes.append(4)
        rem -= 4
    assert rem == 8, nblocks
    sizes += [2, 2, 1, 1, 1, 1]
    return sizes


def _layout(nblocks):
    """Chunk sizes, arena offsets, and reuse gates.

    gates[i] = largest chunk index whose arena region chunk i overwrites
    (-1 if the region is virgin); the DMA for chunk i must wait until PE
    has retired that chunk (sem_mm >= mmcum[gates[i]]).
    """
    sizes = _sched_sizes(nblocks)
    assert sum(sizes) == nblocks
    off, gates = [], []
    owner = [-1] * W
    o = 0
    for i, g in enumerate(sizes):
        assert g <= W
        if o + g > W:
            o = 0
        gate = -1
        for wb in range(o, o + g):
            gate = max(gate, owner[wb])
            owner[wb] = i
        assert gate < i
        off.append(o)
        gates.append(gate)
        o += g
        if o == W:
            o = 0
    return sizes, off, gates


def build(n=N, d=D):
    key = (n, d)
    if key in _cache:
        return _cache[key]

    nblocks = n // P
    assert nblocks * P == n
    nbank = d // 512
    sizes, off, gates = _layout(nblocks)
    nchunk = len(sizes)

    # cumulative counters after each chunk
    sttcum = []
    mmcum = []
    t_s, t_m = 0, 0
    for g in sizes:
        t_s += g
        t_m += g * nbank + 1
        sttcum.append(t_s)
        mmcum.append(t_m)

    nc = bacc.Bacc("TRN2", target_bir_lowering=False, debug=False, num_devices=B)
    feats = nc.declare_dram_parameter("feats", [n, d], F32, isOutput=False)
    weight = nc.declare_dram_parameter("weight", [P, d], F32, isOutput=False)
    out = nc.declare_dram_parameter("out", [1, d], F32, isOutput=True)

    feats_f = feats.ap()
    srcs = []
    r0 = 0
    for g in sizes:
        rows = P * g
        srcs.append(
            feats_f[r0 : r0 + rows, :]
            .rearrange("(p g) d -> p (g d)", g=g)
            .bitcast(F32R)
        )
        r0 += rows

    with contextlib.ExitStack() as ctx:
        arena = ctx.enter_context(nc.sbuf_tensor("arena", [P, W * d], F32R))
        scr = [
            ctx.enter_context(nc.sbuf_tensor(f"scr{k}", [P, d], F32)) for k in range(2)
        ]
        w_bc = ctx.enter_context(nc.sbuf_tensor("w_bc", [P, d], F32))
        gmax = max(sizes)
        s_t = [
            ctx.enter_context(nc.sbuf_tensor(f"s{k}", [P, gmax], F32)) for k in range(S)
        ]
        p_t = [
            ctx.enter_context(nc.sbuf_tensor(f"p{k}", [P, gmax], F32R))
            for k in range(S)
        ]
        zg = [
            ctx.enter_context(nc.sbuf_tensor(f"zg{k}", [P, 1], F32)) for k in range(S)
        ]
        ones = ctx.enter_context(nc.sbuf_tensor("ones", [P, 1], F32))
        rec = ctx.enter_context(nc.sbuf_tensor("rec", [1, 1], F32))
        # final result reuses scr[0]'s partition-0 row (scr is dead by then)
        res = scr[0][0:1, :]
        acc = ctx.enter_context(nc.psum_tensor("acc", [1, d], F32))
        zacc = ctx.enter_context(nc.psum_tensor("zacc", [1, 1], F32))

        # tail chunks are exempt from the L-lead pacing gate (a 1-block tail
        # chunk gated on a 4-block score serializes the stream end); they are
        # released in one burst once DVE has scored through chunk REL
        tail0 = max(L, nchunk - 8)
        rel = max(0, tail0 - 3)

        block = ctx.enter_context(nc.Block(no_gpsimd_drain=True))
        sem_wb = ctx.enter_context(nc.semaphore("sem_wb"))
        sem_c = [ctx.enter_context(nc.semaphore(f"sem_c{i}")) for i in range(nchunk)]
        sem_out = ctx.enter_context(nc.semaphore("sem_out"))
        sem_dve = ctx.enter_context(nc.semaphore("sem_dve"))
        sem_exp = ctx.enter_context(nc.semaphore("sem_exp"))
        sem_mm = ctx.enter_context(nc.semaphore("sem_mm"))
        sem_res = ctx.enter_context(nc.semaphore("sem_res"))
        sem_one = ctx.enter_context(nc.semaphore("sem_one"))

        def chunk_dst(i):
            return arena[:, off[i] * d : (off[i] + sizes[i]) * d]

        def fblk(i, gg):
            return arena[:, (off[i] + gg) * d : (off[i] + gg + 1) * d]

        half = (d // 2 // 512) * 512 or d // 2

        @block.sync
        def _(sync):
            # even chunks; issue paced L chunks ahead of DVE score progress
            # (deep free-running queues measurably unbalance the two rings'
            # HBM arbitration; DVE-paced issue keeps them even)
            sync.dma_start(out=w_bc[:, 0 : d // 2], in_=weight.ap()[:, 0 : d // 2]).then_inc(
                sem_wb, 16
            )
            watermark = -1
            released = False
            for i in range(0, nchunk, 2):
                if i >= tail0:
                    if not released:
                        sync.wait_ge(sem_dve, sttcum[rel])
                        released = True
                elif i >= L:
                    sync.wait_ge(sem_dve, sttcum[i - L])
                if gates[i] > watermark:
                    sync.wait_ge(sem_mm, mmcum[gates[i]])
                    watermark = gates[i]
                sync.dma_start(out=chunk_dst(i), in_=srcs[i]).then_inc(sem_c[i], 16)
            sync.wait_ge(sem_res, 2)
            sync.dma_start(out=out[:], in_=res).then_inc(sem_out, 16)
            sync.wait_ge(sem_out, 16)

        @block.vector
        def _(vector):
            nc.vector.memset(ones[:], 1.0).then_inc(sem_one, 1)
            vector.wait_ge(sem_wb, 32)
            kop = 0
            for i, g in enumerate(sizes):
                vector.wait_ge(sem_c[i], 16)
                if i >= S:
                    vector.wait_ge(sem_exp, i - S + 1)
                s = s_t[i % S]
                for gg in range(g):
                    ins = nc.vector.scalar_tensor_tensor(
                        out=scr[kop % 2][:],
                        in0=fblk(i, gg).bitcast(F32),
                        scalar=1.0,
                        in1=w_bc[:],
                        op0=mybir.AluOpType.mult,
                        op1=mybir.AluOpType.mult,
                        accum_out=s[:, gg : gg + 1],
                    )
                    ins.then_inc(sem_dve, 1)
                    if kop >= 1:
                        ins._wait_ge(sem_dve, kop - 1)
                    kop += 1
            vector.wait_ge(sem_mm, mmcum[-1])
            r1 = nc.vector.reciprocal(rec[:], zacc[:])
            r1.then_inc(sem_dve, 1)
            r1._wait_ge(sem_dve, kop - 1)
            r2 = nc.vector.tensor_scalar_mul(
                res[:, 0:half], acc[:, 0:half], rec[:]
            )
            r2.then_inc(sem_res, 1)
            r2._wait_ge(sem_dve, kop + 1)

        @block.scalar
        def _(scalar):
            scalar.dma_start(
                out=w_bc[:, d // 2 : d], in_=weight.ap()[:, d // 2 : d]
            ).then_inc(sem_wb, 16)
            watermark = -1

            def issue(j):
                nonlocal watermark
                if j % 2 == 1 and j < nchunk:
                    if gates[j] > watermark:
                        scalar.wait_ge(sem_mm, mmcum[gates[j]])
                        watermark = gates[j]
                    scalar.dma_start(out=chunk_dst(j), in_=srcs[j]).then_inc(
                        sem_c[j], 16
                    )

            for j in range(min(L, nchunk)):
                issue(j)
            for i, g in enumerate(sizes):
                scalar.wait_ge(sem_dve, sttcum[i])
                if i >= S:
                    scalar.wait_ge(sem_mm, mmcum[i - S])
                nc.scalar.activation(
                    p_t[i % S][:, 0:g],
                    s_t[i % S][:, 0:g],
                    mybir.ActivationFunctionType.Exp,
                    accum_out=zg[i % S][:],
                ).then_inc(sem_exp, 1)
                # the L-lead gate for chunk i+L is exactly "score(i) done",
                # which the exp above just waited for: zero-stall issue point
                if i + L < tail0:
                    issue(i + L)
                if i == rel:
                    for j in range(tail0, nchunk):
                        issue(j)
            # other half of the final 1/z scaling (DVE computes rec first)
            scalar.wait_ge(sem_dve, sttcum[-1] + 1)
            nc.scalar.activation(
                res[:, half:d],
                acc[:, half:d],
                mybir.ActivationFunctionType.Copy,
                scale=rec[:],
            ).then_inc(sem_res, 1)

        @block.tensor
        def _(tensor):
            tensor.wait_ge(sem_one, 1)
            mop = 0
            for i, g in enumerate(sizes):
                tensor.wait_ge(sem_exp, i + 1)
                p = p_t[i % S]
                for gg in range(g):
                    first = i == 0 and gg == 0
                    last = i == nchunk - 1 and gg == g - 1
                    f = fblk(i, gg)
                    for bk in range(nbank):
                        ins = nc.tensor.matmul(
                            acc[:, bk * 512 : (bk + 1) * 512],
                            p[:, gg : gg + 1],
                            f[:, bk * 512 : (bk + 1) * 512],
                            start=first,
                            stop=last,
                        )
                        ins.then_inc(sem_mm, 1)
                        if mop >= 1:
                            ins._wait_ge(sem_mm, mop - 1)
                        mop += 1
                ins = nc.tensor.matmul(
                    zacc[:],
                    zg[i % S][:],
                    ones[:],
                    start=(i == 0),
                    stop=(i == nchunk - 1),
                )
                ins.then_inc(sem_mm, 1)
                ins._wait_ge(sem_mm, mop - 1)
                mop += 1

    nc.compile()
    _cache[key] = nc
    return nc


def kernel(feats, weight):
    feats = np.ascontiguousarray(np.asarray(feats), dtype=np.float32)
    weight = np.ascontiguousarray(np.asarray(weight), dtype=np.float32)
    assert feats.shape == (B, N, D) and weight.shape == (D,)
    w_rep = np.ascontiguousarray(np.broadcast_to(weight[None, :], (P, D)))
    nc = build()
    in_maps = [
        {"feats": np.ascontiguousarray(feats[b]), "weight": w_rep} for b in range(B)
    ]
    r = run_bass_kernel_spmd(nc, in_maps, core_ids=list(range(B)))
    return np.stack([r.results[b]["out"][0] for b in range(B)], axis=0)


if __name__ == "__main__":
    from concourse.bass_interp import CoreSim

    n_s, d_s = 2048, 1024
    nc = build(n=n_s, d=d_s)
    rng = np.random.default_rng(0)
    f = rng.standard_normal((n_s, d_s), dtype=np.float32)
    w = rng.random(d_s, dtype=np.float32)
    sim = CoreSim(nc, trace=False)
    sim.tensor("feats")[:] = f
    sim.tensor("weight")[:] = np.broadcast_to(w[None, :], (P, d_s))
    sim.simulate(check_with_hw=False)
    got = np.array(sim.tensor("out"))[0]

    s = (f.astype(np.float64) * w.astype(np.float64)).sum(1)
    p = np.exp(s - s.max())
    exp = (p / p.sum()) @ f.astype(np.float64)
    rel = np.abs(got - exp).max() / np.abs(exp).max()
    print("CoreSim rel err:", rel)
    assert rel < 2e-3, rel
    print("SMOKE OK")
